# revision 1
# baseline (speedup 1.0000x reference)
"""Cross-modal attention fusion kernel for Trainium2, SPMD over 8 NeuronCores.

Problem (per batch element b of 16, data-parallel 2 per core):
  q = Wq_rgb@rgb+bq, k = Wk_dep@dep, v = Wv_dep@dep          (1x1 convs)
  rgb_att = softmax(q^T k / sqrt(C)) @ v^T  (and symmetric dep_att)
  fused = W_fuse @ concat(rgb_att, dep_att)
  out = relu(batchnorm_train(fused) * gamma + beta)   (global batch stats)

Distribution: batch elements 2i,2i+1 on core i; BN batch stats via a
(128x16) AllReduce across the 8 cores; weights replicated.

On-device layouts (per batch element; partition dim first):
  inputs rgb/dep      (128, 4cc, 1024n)   ch = cc*128+p
  Q, K                (128, 1024)         dir1 on partitions 0:64, dir2 64:128
  V^T                 (128m, 8mc, 512c)   hw-position m on partitions
  S^T = K^T Q scores  (128m, 1024n)  -> U^T = exp(S^T/sqrt(C))
  softmax denominator: ones-matmul over U^T partitions, PSUM-accumulated
  over the 8 m-chunks (reduce + broadcast across partitions in one)
  att = V^T.T @ U^T   (128c, 4cch, 1024n), normalized by 1/denom
  fused = WfT.T @ cat (128o, 4oc, 1024n); both batches stay in SBUF
  until the stats AllReduce completes, then affine+relu in place

All matmuls run in float32r (single-pass fp32, ~1e-4 rel err measured).
Key-bias and value-bias are dropped: exactly cancelled by softmax
shift-invariance / BN shift-invariance respectively.
PSUM is managed as 8 uniform 1-bank (128,512) slots so banks recycle fast.
"""

import numpy as np

import concourse.bass as bass
import concourse.mybir as mybir
import concourse.tile as tile
from concourse import bacc
from concourse import bass_utils
from concourse.bass import ts

N_CORES = 8
B, C, H, W = 16, 512, 32, 32
HW = H * W          # 1024
CQ = C // 8         # 64
BPC = B // N_CORES  # 2 batch elements per core
EPS = 1e-5
INV_SCALE = 1.0 / float(np.float32(C) ** 0.5)
NB = HW // 512      # free-dim halves per matmul
F32 = mybir.dt.float32
F32R = mybir.dt.float32r
AF = mybir.ActivationFunctionType

_CACHE = {}


def build(n_cores=N_CORES, compile=True, use_collective=True, n_reps=1,
          n_iters=None):
    key = ("nc", n_cores, use_collective, n_reps, n_iters)
    if key in _CACHE:
        return _CACHE[key]
    nc = bacc.Bacc("TRN2", target_bir_lowering=False, debug=False,
                   num_devices=n_cores)

    rgb_d = nc.dram_tensor("rgb", [BPC, C, HW], F32, kind="ExternalInput")
    dep_d = nc.dram_tensor("dep", [BPC, C, HW], F32, kind="ExternalInput")
    # [Wq_rgb; Wk_rgb].T and [Wk_dep; Wq_dep].T — the two 64-row heads that
    # share an input are fused into one M=128 matmul
    wqkr_d = nc.dram_tensor("wqkr", [C, 2 * CQ], F32, kind="ExternalInput")
    wqkd_d = nc.dram_tensor("wqkd", [C, 2 * CQ], F32, kind="ExternalInput")
    wv1t_d = nc.dram_tensor("wv1t", [C, C], F32, kind="ExternalInput")
    wv2t_d = nc.dram_tensor("wv2t", [C, C], F32, kind="ExternalInput")
    wft_d = nc.dram_tensor("wft", [2 * C, C], F32, kind="ExternalInput")
    bq1_d = nc.dram_tensor("bq1", [CQ, 1], F32, kind="ExternalInput")
    bq2_d = nc.dram_tensor("bq2", [CQ, 1], F32, kind="ExternalInput")
    gam_d = nc.dram_tensor("gam", [128, 4], F32, kind="ExternalInput")
    bet_d = nc.dram_tensor("bet", [128, 4], F32, kind="ExternalInput")
    out_d = nc.dram_tensor("out", [BPC, C, HW], F32, kind="ExternalOutput")

    def r128(ap):
        # (X*128, Y) dram -> (128p, Xcc, Y) partition-major view
        return ap.rearrange("(cc p) y -> p cc y", p=128)

    dbg = {}
    _CACHE[("dbg", n_cores)] = dbg

    with tile.TileContext(nc) as tc:
        with (
            tc.tile_pool(name="wp", bufs=1) as wp,
            tc.tile_pool(name="inp", bufs=1) as inp,
            tc.tile_pool(name="qkp", bufs=1) as qkp,
            tc.tile_pool(name="vtp", bufs=1) as vtp,
            tc.tile_pool(name="utp", bufs=8) as utp,
            tc.tile_pool(name="rip", bufs=2) as rip,
            tc.tile_pool(name="atp", bufs=2) as atp,
            tc.tile_pool(name="fup", bufs=2) as fup,
            tc.tile_pool(name="smp", bufs=1) as smp,
            tc.tile_pool(name="psp", bufs=8, space="PSUM") as psp,
            tc.tile_pool(name="drp", bufs=1, space="DRAM") as drp,
        ):
            # ---- small weights / constants first (unblock first matmuls) ----
            # two HWDGE queues: nc.sync (SP) and nc.scalar (Activation)
            wqkr = wp.tile([128, 4, 2 * CQ], F32R)
            wqkd = wp.tile([128, 4, 2 * CQ], F32R)
            nc.sync.dma_start(wqkr[:], r128(wqkr_d[:]).bitcast(F32R))
            nc.scalar.dma_start(wqkd[:], r128(wqkd_d[:]).bitcast(F32R))
            bq1 = wp.tile([CQ, 1], F32)
            bq2 = wp.tile([2 * CQ, 1], F32)  # bq2 lives on partitions 64:128
            nc.sync.dma_start(bq1[:], bq1_d[:])
            nc.scalar.dma_start(bq2[CQ:2 * CQ, :], bq2_d[:])
            gam = wp.tile([128, 4], F32)
            bet = wp.tile([128, 4], F32)
            nc.sync.dma_start(gam[:], gam_d[:])
            nc.scalar.dma_start(bet[:], bet_d[:])
            ones_f = wp.tile([128, 128], F32)
            nc.vector.memset(ones_f[:], 1.0)
            ones = wp.tile([128, 128], F32R)
            nc.vector.tensor_copy(ones[:], ones_f[:])
            eps_t = wp.tile([128, 1], F32)
            nc.vector.memset(eps_t[:], EPS)
            # dummy Ln: pins the natural_log_exp_and_others ACT table set,
            # which covers every func used here (exp/ln/copy/square/relu) ->
            # zero mid-kernel table reloads
            lnw = wp.tile([128, 1], F32)
            nc.scalar.activation(out=lnw[:], in_=eps_t[:], func=AF.Ln)
            # big weights: tiles declared here, DMAs emitted after batch-0
            # inputs so the input stream isn't stuck behind 4.5 MB of weights
            wv1t = wp.tile([128, 4, C], F32R)
            wv2t = wp.tile([128, 4, C], F32R)
            wft = wp.tile([128, 8, C], F32R)

            def one_rep():
              # body emitted n_reps times (timing builds use n_reps=2 and
              # measure the wall-clock delta to isolate device time)
              ssum = smp.tile([128, 16], F32, tag="ssum", name="ssum")
              ssq = smp.tile([128, 16], F32, tag="ssq", name="ssq")

              def qk_mm(wt, src, outs):
                  # outs: [(dst, dst_lo, src_lo, bias_t or None), ...] row splits
                  ph = [psp.tile([128, 512], F32, tag="ps", name="ps_qk")
                        for _ in range(NB)]
                  for cc in range(4):
                      for nh in range(NB):
                          nc.tensor.matmul(
                              ph[nh][:], wt[:, cc, :],
                              src[:, cc, ts(nh, 512)],
                              start=(cc == 0), stop=(cc == 3))
                  for nh in range(NB):
                      for dst, lo, slo, bias_t in outs:
                          if bias_t is None:
                              nc.vector.tensor_copy(
                                  dst[lo:lo + CQ, ts(nh, 512)],
                                  ph[nh][slo:slo + CQ, :])
                          else:
                              nc.vector.tensor_scalar_add(
                                  dst[lo:lo + CQ, ts(nh, 512)],
                                  ph[nh][slo:slo + CQ, :], bias_t)

              # ---------------- per batch element ----------------
              fus = []
              for b in range(BPC):
                  rgb_sb = inp.tile([128, 4, HW], F32R, tag="rgb", name="rgb_sb")
                  dep_sb = inp.tile([128, 4, HW], F32R, tag="dep", name="dep_sb")
                  for cc in range(4):
                      nc.sync.dma_start(rgb_sb[:, cc, :],
                                        r128(rgb_d[b]).bitcast(F32R)[:, cc, :])
                      nc.scalar.dma_start(dep_sb[:, cc, :],
                                          r128(dep_d[b]).bitcast(F32R)[:, cc, :])
                  if b == 0:
                      nc.sync.dma_start(wv1t[:], r128(wv1t_d[:]).bitcast(F32R))
                      nc.scalar.dma_start(wv2t[:], r128(wv2t_d[:]).bitcast(F32R))
                      wftv = r128(wft_d[:]).bitcast(F32R)
                      nc.sync.dma_start(wft[:, 0:4, :], wftv[:, 0:4, :])
                      nc.scalar.dma_start(wft[:, 4:8, :], wftv[:, 4:8, :])

                  # Q/K for both directions: dir1 on partitions 0:64, dir2 64:128
                  qA = qkp.tile([128, HW], F32R, tag="qA", name="qA")
                  kA = qkp.tile([128, HW], F32R, tag="kA", name="kA")
                  qk_mm(wqkr, rgb_sb, [(qA, 0, 0, bq1[:]),
                                       (kA, CQ, CQ, None)])
                  qk_mm(wqkd, dep_sb, [(kA, 0, 0, None),
                                       (qA, CQ, CQ, bq2[CQ:2 * CQ, :])])
                  dbg[("qA", b)] = qA.name
                  dbg[("kA", b)] = kA.name

                  atts = []
                  for d, (vsrc, wvt) in enumerate(
                      [(dep_sb, wv1t), (rgb_sb, wv2t)]
                  ):
                      lo = d * CQ
                      q = qA[lo:lo + CQ, :]
                      k = kA[lo:lo + CQ, :]
                      # V^T: (128m, 8mc, 512c)
                      vt = vtp.tile([128, 8, C], F32R, tag="vt", name="vt")
                      for m in range(8):
                          ps = psp.tile([128, C], F32, tag="ps", name="ps_vt")
                          for cc in range(4):
                              nc.tensor.matmul(
                                  ps[:], vsrc[:, cc, ts(m, 128)], wvt[:, cc, :],
                                  start=(cc == 0), stop=(cc == 3))
                          nc.vector.tensor_copy(vt[:, m, :], ps[:])

                      # S^T -> U^T = exp(S^T/sqrt(C))
                      uts = []
                      for m in range(8):
                          ph = [psp.tile([128, 512], F32, tag="ps", name="ps_st")
                                for _ in range(NB)]
                          for nh in range(NB):
                              nc.tensor.matmul(
                                  ph[nh][:], k[:, ts(m, 128)],
                                  q[:, ts(nh, 512)], start=True, stop=True)
                          ut = utp.tile([128, HW], F32R, tag="ut", name="ut")
                          for nh in range(NB):
                              nc.scalar.activation(
                                  out=ut[:, ts(nh, 512)], in_=ph[nh][:],
                                  func=AF.Exp, scale=INV_SCALE)
                          uts.append(ut)

                      # denominator: PSUM-accumulated ones-matmul over U^T
                      # partition axis (reduce + broadcast in one); PE-only,
                      # both DVE and GpSimd measured slower here on HW
                      rs = [psp.tile([128, 512], F32, tag="ps", name="ps_rs")
                            for _ in range(NB)]
                      for m in range(8):
                          for nh in range(NB):
                              nc.tensor.matmul(
                                  rs[nh][:], ones[:], uts[m][:, ts(nh, 512)],
                                  start=(m == 0), stop=(m == 7))
                      rinv = rip.tile([128, HW], F32, tag="ri", name="rinv")
                      for nh in range(NB):
                          nc.vector.reciprocal(rinv[:, ts(nh, 512)], rs[nh][:])

                      # att = (V^T)^T @ U^T, normalized
                      att = atp.tile([128, 4, HW], F32R, tag="att", name="att")
                      for cch in range(4):
                          ph = [psp.tile([128, 512], F32, tag="ps", name="ps_pv")
                                for _ in range(NB)]
                          for m in range(8):
                              for nh in range(NB):
                                  nc.tensor.matmul(
                                      ph[nh][:], vt[:, m, ts(cch, 128)],
                                      uts[m][:, ts(nh, 512)],
                                      start=(m == 0), stop=(m == 7))
                          for nh in range(NB):
                              nc.vector.tensor_mul(att[:, cch, ts(nh, 512)],
                                                   ph[nh][:],
                                                   rinv[:, ts(nh, 512)])
                      atts.append(att)
                      dbg[("vt", b, d)] = vt.name
                      dbg[("ut", b, d)] = [u.name for u in uts]
                      dbg[("rinv", b, d)] = rinv.name
                      dbg[("att", b, d)] = att.name

                  # fused = WfT.T @ [att1; att2]; stats + SBUF staging
                  fu = fup.tile([128, 4, HW], F32, tag="fu", name="fu")
                  fus.append(fu)
                  for o in range(4):
                      ph = [psp.tile([128, 512], F32, tag="ps", name="ps_f")
                            for _ in range(NB)]
                      for kc in range(8):
                          src = atts[kc // 4]
                          for nh in range(NB):
                              nc.tensor.matmul(
                                  ph[nh][:], wft[:, kc, ts(o, 128)],
                                  src[:, kc % 4, ts(nh, 512)],
                                  start=(kc == 0), stop=(kc == 7))
                      for nh in range(NB):
                          col = (b * 2 + nh) * 4 + o
                          nc.scalar.activation(
                              out=fu[:, o, ts(nh, 512)], in_=ph[nh][:],
                              func=AF.Copy, accum_out=ssum[:, col:col + 1])
                          # sumsq: square the psum half in place, accumulate
                          nc.scalar.activation(
                              out=ph[nh][:], in_=ph[nh][:], func=AF.Square,
                              accum_out=ssq[:, col:col + 1])

              # ---------------- global BN stats ----------------
              t8s = smp.tile([128, 8], F32)
              t8q = smp.tile([128, 8], F32)
              nc.vector.tensor_add(t8s[:], ssum[:, 0:8], ssum[:, 8:16])
              nc.vector.tensor_add(t8q[:], ssq[:, 0:8], ssq[:, 8:16])
              tot = smp.tile([128, 8], F32)
              nc.vector.tensor_add(tot[:, 0:4], t8s[:, 0:4], t8s[:, 4:8])
              nc.vector.tensor_add(tot[:, 4:8], t8q[:, 0:4], t8q[:, 4:8])
              cc_in = drp.tile([128, 8], F32)
              cc_out = drp.tile([128, 8], F32)
              nc.sync.dma_start(cc_in[:], tot[:])
              if use_collective:
                  nc.gpsimd.collective_compute(
                      "AllReduce", mybir.AluOpType.add,
                      replica_groups=[list(range(n_cores))],
                      ins=[cc_in.opt()], outs=[cc_out.opt()])
              else:
                  nc.sync.dma_start(cc_out[:], cc_in[:])
              gst = smp.tile([128, 8], F32)
              nc.sync.dma_start(gst[:], cc_out[:])

              inv_n = 1.0 / float(B * HW)
              ms = smp.tile([128, 8], F32)
              nc.vector.tensor_scalar_mul(ms[:], gst[:], inv_n)
              mean = ms[:, 0:4]
              var = smp.tile([128, 4], F32)
              nc.vector.tensor_mul(var[:], mean, mean)
              nc.vector.tensor_sub(var[:], ms[:, 4:8], var[:])
              # rstd = exp(-0.5*ln(var+eps))  (ln+exp share one ACT table set)
              lnv = smp.tile([128, 4], F32)
              nc.scalar.activation(out=lnv[:], in_=var[:], func=AF.Ln,
                                   bias=eps_t[:])
              rstd = smp.tile([128, 4], F32)
              nc.scalar.activation(out=rstd[:], in_=lnv[:], func=AF.Exp,
                                   scale=-0.5)
              a_t = smp.tile([128, 4], F32)
              b_t = smp.tile([128, 4], F32)
              nc.vector.tensor_mul(a_t[:], rstd[:], gam[:])
              nc.vector.tensor_mul(b_t[:], mean[:], a_t[:])
              nc.vector.tensor_sub(b_t[:], bet[:], b_t[:])
              dbg.update(fus=[f.name for f in fus], ssum=ssum.name,
                         ssq=ssq.name, tot=tot.name, gst=gst.name,
                         mean=mean.name, var=var.name, rstd=rstd.name,
                         a_t=a_t.name, b_t=b_t.name)

              # ---------------- apply + writeback (both batches in SBUF) ----
              # split the affine+relu between ScalarE (1 pass) and VectorE
              # (2 passes) so neither engine serializes the tail
              for b in range(BPC):
                  for o in range(4):
                      fu = fus[b]
                      for nh in range(NB):
                          dst = fu[:, o, ts(nh, 512)]
                          if (o * NB + nh) % 3 == 2:
                              nc.vector.tensor_scalar(
                                  out=dst, in0=dst, scalar1=a_t[:, o:o + 1],
                                  scalar2=b_t[:, o:o + 1],
                                  op0=mybir.AluOpType.mult,
                                  op1=mybir.AluOpType.add)
                              nc.vector.tensor_scalar_max(dst, dst, 0.0)
                          else:
                              nc.scalar.activation(
                                  out=dst, in_=dst, func=AF.Relu,
                                  scale=a_t[:, o:o + 1], bias=b_t[:, o:o + 1])
                          q = nc.sync if (o + nh) % 2 == 0 else nc.scalar
                          q.dma_start(out_d[b, ts(o, 128), ts(nh, 512)], dst)

            if n_iters is not None:
                with tc.For_i(0, n_iters, 1):
                    one_rep()
            else:
                for _rep in range(n_reps):
                    one_rep()

    if compile:
        nc.compile()
    _CACHE[key] = nc
    return nc


def kernel(rgb, depth, Wq_rgb, bq_rgb, Wk_dep, bk_dep, Wv_dep, bv_dep,
           Wq_dep, bq_dep, Wk_rgb, bk_rgb, Wv_rgb, bv_rgb, W_fuse,
           gamma, beta):
    nc = build()

    def f32c(x):
        return np.ascontiguousarray(np.asarray(x), dtype=np.float32)

    rgb_f = f32c(rgb).reshape(B, C, HW)
    dep_f = f32c(depth).reshape(B, C, HW)
    shared = {
        "wqkr": f32c(np.concatenate([np.asarray(Wq_rgb),
                                     np.asarray(Wk_rgb)], axis=0).T),
        "wqkd": f32c(np.concatenate([np.asarray(Wk_dep),
                                     np.asarray(Wq_dep)], axis=0).T),
        "wv1t": f32c(np.asarray(Wv_dep).T),
        "wv2t": f32c(np.asarray(Wv_rgb).T),
        "wft": f32c(np.asarray(W_fuse).T),
        "bq1": f32c(bq_rgb).reshape(CQ, 1),
        "bq2": f32c(bq_dep).reshape(CQ, 1),
        "gam": f32c(np.asarray(gamma).reshape(4, 128).T),
        "bet": f32c(np.asarray(beta).reshape(4, 128).T),
    }
    in_maps = []
    for i in range(N_CORES):
        m = dict(shared)
        m["rgb"] = rgb_f[BPC * i:BPC * (i + 1)]
        m["dep"] = dep_f[BPC * i:BPC * (i + 1)]
        in_maps.append(m)

    res = bass_utils.run_bass_kernel_spmd(
        nc, in_maps, core_ids=list(range(N_CORES)))
    out = np.concatenate(
        [res.results[i]["out"].reshape(BPC, C, H, W) for i in range(N_CORES)],
        axis=0)
    return out.astype(np.float32)

